# revision 21
# baseline (speedup 1.0000x reference)
"""Trainium2 Bass kernel for BinaryRelativePositionEmbedding.

Math: out[b,h,l,m] = q[b,h,l,:] . rp[m,:],  rp = bits @ emb, where
bits[m,:] are the 12 two's-complement bits of position (m - L + 1).

Key identity: out[l, m] = sum_b bits[m,b] * s[l,b] with s = q @ emb^T
(rank 12).  The pattern v(m) = (m - (L-1)) & 4095 ranges over all 12-bit
values except 2048, so each row-tile of the output is a subset-sum table
over the 12 per-row scalars s[l, :], built with doubling steps on the
vector engine.  The table is laid out rotated by 2048 so the final
output row is the single contiguous slice U[:, 1:4096]:
    U[:, 2048+w] = subset-sum of bits 0..10 over w   (w in [0,2048))
    U[:, c]      = U[:, 2048+c] + s_11               (c in [0,2048))
    => U[:, 1+m] = T[(m + 2049) & 4095] = out[:, m]  (m in [0,4095))
giving one 16380-byte contiguous DMA descriptor per output row.

All output batches go on the single sync HWDGE ring (inputs on the
scalar ring): with exactly one deeply-backed queue the 16 SDMA engines
drain it at ~26.7 GB/s each (~427 GB/s total, 612 ns per 16380-byte
descriptor — the SBUF-AXI per-engine ceiling).  Two concurrently-busy
queues make every engine round-robin at packet granularity and drop to
~19.7 GB/s each.  No SDMA-engine degradation over a 340 us single-ring
run.  The steady state is bistable: if the DVE builds stay ~1-2 batches
ahead of the drain (they do, 7.4 us/batch vs 9.9), dispatches always
land before the ring empties; perturbing the ramp so the DMA catches up
with the builds locks in a ~12.2 us/batch starving mode (-60 us).  The
2-tile batch structure is ALSO load-bearing: the Tile scheduler
interleaves the two tiles' build chains on the DVE, hiding each op's
completion-semaphore latency (~40% of op duration); single-tile batches
or forced serial order drop DVE throughput to 6-6.6 us/tile and make it
the pacer.  U pool bufs=3 is the max that fits SBUF (bufs=4 silently
corrupts results).

Sharding: data-parallel over the 32 (b,h) pairs, 4 per NeuronCore.
Measured: ~345-350 us/core (output-DMA roofline ~314 us + ~21 us ramp
+ ~4 us completion tail); session baseline was ~423 us.
"""

import os
import sys

import numpy as np

if "/opt/trn_rl_repo" not in sys.path:
    sys.path.insert(0, "/opt/trn_rl_repo")

import concourse.bass as bass  # noqa: E402
import concourse.mybir as mybir  # noqa: E402
from concourse import bacc, tile  # noqa: E402
from concourse.bass_utils import run_bass_kernel_spmd  # noqa: E402

F32 = mybir.dt.float32

B, H, L, D = 2, 16, 2048, 64
NB = 12                  # bits per position
M = 2 * L - 1            # 4095 relative positions
NCORES = 8
PAIRS = B * H            # 32
PPC = PAIRS // NCORES    # 4 (b,h) pairs per core
ROWS = PPC * L           # 8192 output rows per core


LAST_EXEC_TIME_NS = None


def _build_nc():
    nc = bacc.Bacc(None)
    # qTe packs embT (cols 0:12) ahead of qT (cols 12:12+ROWS) so the first
    # chunk load delivers both with a single DMA dispatch + completion
    # (a separate 3 KB embT DMA costs ~3 us of serial ramp).
    qTe = nc.declare_dram_parameter("qTe", [D, NB + ROWS], F32, isOutput=False)
    out = nc.declare_dram_parameter("out", [ROWS, M], F32, isOutput=True)

    nt = ROWS // 128  # 64 row-tiles of 128 rows

    # Graduated input chunks (col ranges of qTe): a tiny first chunk so the
    # first matmul + table build can start early, then larger ones.  All
    # chunks are [64, csz] (8 DMA ports, ~190 GB/s) — the total 2 MB load
    # fully overlaps the early output batches.
    chunk_bounds = [(0, 268), (268, 1036), (1036, 2060),
                    (2060, 4108), (4108, 6156), (6156, 8204)]
    # matmul/copy groups (tile ranges), each within a single chunk; group 0
    # is a single tile so the first PSUM copy lands as early as possible.
    groups = [(0, 1), (1, 2), (2, 8), (8, 16), (16, 24), (24, 32),
              (32, 40), (40, 48), (48, 56), (56, 64)]

    def chunk_of(t):
        for ci, (a, b) in enumerate(chunk_bounds):
            if a <= NB + t * 128 < b:
                return ci
        raise AssertionError

    # output batches: two single-tile batches first so the first bytes hit
    # HBM as soon as tile 0's table exists, then 2-tile batches.
    batches = [[0], [1]] + [[t, t + 1] for t in range(2, nt, 2)]

    with tile.TileContext(nc) as tc:
        with (
            tc.tile_pool(name="const", bufs=1) as cpool,
            tc.tile_pool(name="psum", bufs=3, space="PSUM") as ppool,
            tc.tile_pool(name="tab", bufs=3) as tpool,
        ):
            s_sb = cpool.tile([128, nt * NB], F32)
            scr = cpool.tile([128, 1], F32, name="scr", tag="scr")
            qt_chunks = [
                cpool.tile([D, b - a], F32, name=f"qt{ci}", tag=f"qt{ci}")
                for ci, (a, b) in enumerate(chunk_bounds)
            ]
            embt_sb = qt_chunks[0][:, 0:NB]

            # All inputs on the scalar ring: the sync ring is reserved for
            # output batches so its queue is the only deep backlog and the
            # SDMA engines never round-robin between two busy queues
            # (measured ~33% per-descriptor penalty when they do).
            for ci, (a, b) in enumerate(chunk_bounds):
                nc.scalar.dma_start(out=qt_chunks[ci][:], in_=qTe[:, a:b])

            # s[l, b] = q[l, :] . emb[b, :].  PSUM->SBUF copies go on the
            # ACT engine so the DVE queue is pure table builds.
            for gi, (g0, g1) in enumerate(groups):
                ng = g1 - g0
                ps = ppool.tile([128, 8 * NB], F32, name="ps", tag="ps")
                for j, t in enumerate(range(g0, g1)):
                    ci = chunk_of(t)
                    off = NB + t * 128 - chunk_bounds[ci][0]
                    nc.tensor.matmul(
                        ps[:, j * NB : (j + 1) * NB],
                        lhsT=qt_chunks[ci][:, off : off + 128],
                        rhs=embt_sb,
                        start=True,
                        stop=True,
                    )
                nc.scalar.copy(
                    out=s_sb[:, g0 * NB : g1 * NB],
                    in_=ps[:, : ng * NB],
                )

            # NOTE: the Tile scheduler interleaves ~2 adjacent tile build
            # chains on the DVE; this hides each op's completion-semaphore
            # latency (~40% of op duration) behind the other chain's ops.
            # Do NOT force strict per-tile ordering — it drops DVE
            # throughput from 3.7 to 6.6 us/tile and makes DVE the pacer.
            # The steady state is bistable: if the output stream ever
            # catches up with the builds, every dispatch arrives ~2.3 us
            # after the ring empties and the loop locks at 12.2 us/batch
            # (~315 GB/s) instead of 9.85 (~427).  Which mode a run lands
            # in is a ramp-timing race — so force it: hold the first two
            # dispatches until batch 2's table is built (a zero-add through
            # scr into a DMA-read column makes it a real data dependency),
            # guaranteeing the builds a self-sustaining 2-batch lead.
            pend = []
            for b, batch in enumerate(batches):
                nb = len(batch)
                U = tpool.tile([128, 2 * 4096], F32, name="U", tag="U")
                for j, ti in enumerate(batch):
                    sb = ti * NB
                    base = j * 4096
                    hi = base + 2048
                    nc.vector.memset(U[:, hi : hi + 1], 0.0)
                    nc.vector.tensor_copy(
                        out=U[:, hi + 1 : hi + 2], in_=s_sb[:, sb : sb + 1]
                    )
                    for k in range(1, NB - 1):
                        nc.vector.tensor_scalar_add(
                            U[:, hi + 2**k : hi + 2 ** (k + 1)],
                            U[:, hi : hi + 2**k],
                            s_sb[:, sb + k : sb + k + 1],
                        )
                    nc.vector.tensor_scalar_add(
                        U[:, base : base + 2048],
                        U[:, hi : hi + 2048],
                        s_sb[:, sb + NB - 1 : sb + NB],
                    )
                r0 = batch[0] * 128
                src = U[:, : nb * 4096].rearrange("p (j c) -> p j c", j=nb)[
                    :, :, 1:4096
                ]
                dst = out[r0 : r0 + nb * 128, :].rearrange("(j p) m -> p j m", p=128)
                if b < 2:
                    pend.append((dst, src, U))
                    continue
                if pend:
                    # scr = 0 * (batch 2's table) -> release the held batches
                    nc.vector.tensor_scalar_mul(scr[:, 0:1], U[:, 0:1], 0.0)
                    for dstp, srcp, Up in pend:
                        nc.vector.tensor_scalar_add(
                            Up[:, 1:2], Up[:, 1:2], scr[:, 0:1]
                        )
                        nc.sync.dma_start(out=dstp, in_=srcp)
                    pend = []
                # single ring: FIFO order means the engines always drain one
                # queue with deep backlog — measured 26.7 GB/s/engine (427
                # total) vs 19.7 when two queues are concurrently busy.
                nc.sync.dma_start(out=dst, in_=src)

    nc.finalize()
    return nc


def _install_trace_shim():
    """Make run_bass_kernel_spmd(trace=True) work under axon in this
    container: provide antenv.axon_hooks backed by ctypes calls into
    libaxon_pjrt.so, and skip the S3 artifact upload."""
    import contextlib
    import ctypes
    import types

    import antenv
    from concourse import bass_utils

    if getattr(antenv, "axon_hooks", None) is not None:
        return

    def _ntff_profile_via_ctypes(so_path):
        lib = ctypes.CDLL(so_path)
        if not hasattr(lib, "axon_start_nrt_profile"):
            return None
        lib.axon_start_nrt_profile.argtypes = [
            ctypes.POINTER(ctypes.c_int64),
            ctypes.c_size_t,
        ]
        lib.axon_start_nrt_profile.restype = ctypes.c_int64
        lib.axon_stop_nrt_profile.argtypes = [ctypes.c_char_p]
        lib.axon_stop_nrt_profile.restype = ctypes.c_int64

        @contextlib.contextmanager
        def _hook(output_dir, device_ids):
            import jax

            jax.devices()
            if device_ids:
                ids = (ctypes.c_int64 * len(device_ids))(*device_ids)
                rc = lib.axon_start_nrt_profile(ids, len(device_ids))
            else:
                rc = lib.axon_start_nrt_profile(None, 0)
            if rc != 0:
                raise RuntimeError(f"axon_start_nrt_profile rc={rc}")
            try:
                yield
            finally:
                n = lib.axon_stop_nrt_profile(str(output_dir).encode())
                print(f"trace shim: {n} ntff file(s) in {output_dir}", file=sys.stderr)

        return _hook

    mod = types.ModuleType("antenv.axon_hooks")
    state = {"hook": _ntff_profile_via_ctypes("/opt/axon/libaxon_pjrt.so")}
    mod.set_axon_ntff_profile_hook = lambda h: state.__setitem__("hook", h)
    mod.get_axon_ntff_profile_hook = lambda: state["hook"]
    sys.modules["antenv.axon_hooks"] = mod
    antenv.axon_hooks = mod
    bass_utils.upload_artifacts = lambda tmpdir: f"local://{tmpdir}"


def kernel(q, k, emb):
    global LAST_EXEC_TIME_NS
    trace = os.environ.get("KERNEL_TRACE", "") == "1"
    if trace:
        _install_trace_shim()

    nc = _build_nc()

    qr = np.asarray(q, dtype=np.float32).reshape(PAIRS, L, D)
    embT = np.asarray(emb, dtype=np.float32).T  # [D, NB]
    in_maps = []
    for c in range(NCORES):
        qc = qr[c * PPC : (c + 1) * PPC]  # [PPC, L, D]
        qTc = qc.transpose(2, 0, 1).reshape(D, ROWS)
        qTe = np.ascontiguousarray(np.concatenate([embT, qTc], axis=1))
        in_maps.append({"qTe": qTe})

    res = run_bass_kernel_spmd(nc, in_maps, core_ids=list(range(NCORES)), trace=trace)
    LAST_EXEC_TIME_NS = res.exec_time_ns

    out = np.empty((PAIRS, L, M), np.float32)
    for c in range(NCORES):
        out[c * PPC : (c + 1) * PPC] = res.results[c]["out"].reshape(PPC, L, M)
    return out.reshape(B, H, L, M)



# revision 22
# speedup vs baseline: 1.0024x; 1.0024x over previous
"""Trainium2 Bass kernel for BinaryRelativePositionEmbedding.

Math: out[b,h,l,m] = q[b,h,l,:] . rp[m,:],  rp = bits @ emb, where
bits[m,:] are the 12 two's-complement bits of position (m - L + 1).

Key identity: out[l, m] = sum_b bits[m,b] * s[l,b] with s = q @ emb^T
(rank 12).  The pattern v(m) = (m - (L-1)) & 4095 ranges over all 12-bit
values except 2048, so each row-tile of the output is a subset-sum table
over the 12 per-row scalars s[l, :], built with doubling steps on the
vector engine.  The table is laid out rotated by 2048 so the final
output row is the single contiguous slice U[:, 1:4096]:
    U[:, 2048+w] = subset-sum of bits 0..10 over w   (w in [0,2048))
    U[:, c]      = U[:, 2048+c] + s_11               (c in [0,2048))
    => U[:, 1+m] = T[(m + 2049) & 4095] = out[:, m]  (m in [0,4095))
giving one 16380-byte contiguous DMA descriptor per output row.

All output batches go on the single sync HWDGE ring (inputs on the
scalar ring): with exactly one deeply-backed queue the 16 SDMA engines
drain it at ~26.7 GB/s each (~427 GB/s total, 612 ns per 16380-byte
descriptor — the SBUF-AXI per-engine ceiling).  Two concurrently-busy
queues make every engine round-robin at packet granularity and drop to
~19.7 GB/s each.  No SDMA-engine degradation over a 340 us single-ring
run.  The steady state is bistable: if the DVE builds stay ~1-2 batches
ahead of the drain (they do, 7.4 us/batch vs 9.9), dispatches always
land before the ring empties; perturbing the ramp so the DMA catches up
with the builds locks in a ~12.2 us/batch starving mode (-60 us).  The
2-tile batch structure is ALSO load-bearing: the Tile scheduler
interleaves the two tiles' build chains on the DVE, hiding each op's
completion-semaphore latency (~40% of op duration); single-tile batches
or forced serial order drop DVE throughput to 6-6.6 us/tile and make it
the pacer.  U pool bufs=3 is the max that fits SBUF (bufs=4 silently
corrupts results).

Sharding: data-parallel over the 32 (b,h) pairs, 4 per NeuronCore.
Measured: ~345-350 us/core (output-DMA roofline ~314 us + ~21 us ramp
+ ~4 us completion tail); session baseline was ~423 us.
"""

import os
import sys

import ml_dtypes
import numpy as np

if "/opt/trn_rl_repo" not in sys.path:
    sys.path.insert(0, "/opt/trn_rl_repo")

import concourse.bass as bass  # noqa: E402
import concourse.mybir as mybir  # noqa: E402
from concourse import bacc, tile  # noqa: E402
from concourse.bass_utils import run_bass_kernel_spmd  # noqa: E402

F32 = mybir.dt.float32
BF16 = mybir.dt.bfloat16

B, H, L, D = 2, 16, 2048, 64
NB = 12                  # bits per position
M = 2 * L - 1            # 4095 relative positions
NCORES = 8
PAIRS = B * H            # 32
PPC = PAIRS // NCORES    # 4 (b,h) pairs per core
ROWS = PPC * L           # 8192 output rows per core


LAST_EXEC_TIME_NS = None


def _build_nc():
    nc = bacc.Bacc(None)
    # qTe packs embT (cols 0:12) ahead of qT (cols 12:12+ROWS) so the first
    # chunk load delivers both with a single DMA dispatch + completion
    # (a separate 3 KB embT DMA costs ~3 us of serial ramp).
    qTe = nc.declare_dram_parameter("qTe", [D, NB + ROWS], BF16, isOutput=False)
    out = nc.declare_dram_parameter("out", [ROWS, M], F32, isOutput=True)

    nt = ROWS // 128  # 64 row-tiles of 128 rows

    # Graduated input chunks (col ranges of qTe): a tiny first chunk so the
    # first matmul + table build can start early, then larger ones.  All
    # chunks are [64, csz] (8 DMA ports, ~190 GB/s) — the total 2 MB load
    # fully overlaps the early output batches.
    chunk_bounds = [(0, 268), (268, 1036), (1036, 2060),
                    (2060, 4108), (4108, 6156), (6156, 8204)]
    # matmul/copy groups (tile ranges), each within a single chunk; group 0
    # is a single tile so the first PSUM copy lands as early as possible.
    groups = [(0, 1), (1, 2), (2, 8), (8, 16), (16, 24), (24, 32),
              (32, 40), (40, 48), (48, 56), (56, 64)]

    def chunk_of(t):
        for ci, (a, b) in enumerate(chunk_bounds):
            if a <= NB + t * 128 < b:
                return ci
        raise AssertionError

    # output batches: two single-tile batches first so the first bytes hit
    # HBM as soon as tile 0's table exists, then 2-tile batches.
    batches = [[0], [1]] + [[t, t + 1] for t in range(2, nt, 2)]

    with tile.TileContext(nc) as tc:
        with (
            tc.tile_pool(name="const", bufs=1) as cpool,
            tc.tile_pool(name="psum", bufs=3, space="PSUM") as ppool,
            tc.tile_pool(name="tab", bufs=4) as tpool,
        ):
            s_sb = cpool.tile([128, nt * NB], F32)
            scr = cpool.tile([128, 1], F32, name="scr", tag="scr")
            qt_chunks = [
                cpool.tile([D, b - a], BF16, name=f"qt{ci}", tag=f"qt{ci}")
                for ci, (a, b) in enumerate(chunk_bounds)
            ]
            embt_sb = qt_chunks[0][:, 0:NB]

            # All inputs on the scalar ring: the sync ring is reserved for
            # output batches so its queue is the only deep backlog and the
            # SDMA engines never round-robin between two busy queues
            # (measured ~33% per-descriptor penalty when they do).
            for ci, (a, b) in enumerate(chunk_bounds):
                nc.scalar.dma_start(out=qt_chunks[ci][:], in_=qTe[:, a:b])

            # s[l, b] = q[l, :] . emb[b, :].  PSUM->SBUF copies go on the
            # ACT engine so the DVE queue is pure table builds.
            for gi, (g0, g1) in enumerate(groups):
                ng = g1 - g0
                ps = ppool.tile([128, 8 * NB], F32, name="ps", tag="ps")
                for j, t in enumerate(range(g0, g1)):
                    ci = chunk_of(t)
                    off = NB + t * 128 - chunk_bounds[ci][0]
                    nc.tensor.matmul(
                        ps[:, j * NB : (j + 1) * NB],
                        lhsT=qt_chunks[ci][:, off : off + 128],
                        rhs=embt_sb,
                        start=True,
                        stop=True,
                    )
                nc.scalar.copy(
                    out=s_sb[:, g0 * NB : g1 * NB],
                    in_=ps[:, : ng * NB],
                )

            # NOTE: the Tile scheduler interleaves ~2 adjacent tile build
            # chains on the DVE; this hides each op's completion-semaphore
            # latency (~40% of op duration) behind the other chain's ops.
            # Do NOT force strict per-tile ordering — it drops DVE
            # throughput from 3.7 to 6.6 us/tile and makes DVE the pacer.
            # The steady state is bistable: if the output stream ever
            # catches up with the builds, every dispatch arrives ~2.3 us
            # after the ring empties and the loop locks at 12.2 us/batch
            # (~315 GB/s) instead of 9.85 (~427).  Which mode a run lands
            # in is a ramp-timing race — so force it: hold the first two
            # dispatches until batch 2's table is built (a zero-add through
            # scr into a DMA-read column makes it a real data dependency),
            # guaranteeing the builds a self-sustaining 2-batch lead.
            pend = []
            for b, batch in enumerate(batches):
                nb = len(batch)
                U = tpool.tile([128, 2 * 4096], F32, name="U", tag="U")
                for j, ti in enumerate(batch):
                    sb = ti * NB
                    base = j * 4096
                    hi = base + 2048
                    nc.vector.memset(U[:, hi : hi + 1], 0.0)
                    nc.vector.tensor_copy(
                        out=U[:, hi + 1 : hi + 2], in_=s_sb[:, sb : sb + 1]
                    )
                    for k in range(1, NB - 1):
                        nc.vector.tensor_scalar_add(
                            U[:, hi + 2**k : hi + 2 ** (k + 1)],
                            U[:, hi : hi + 2**k],
                            s_sb[:, sb + k : sb + k + 1],
                        )
                    nc.vector.tensor_scalar_add(
                        U[:, base : base + 2048],
                        U[:, hi : hi + 2048],
                        s_sb[:, sb + NB - 1 : sb + NB],
                    )
                r0 = batch[0] * 128
                src = U[:, : nb * 4096].rearrange("p (j c) -> p j c", j=nb)[
                    :, :, 1:4096
                ]
                dst = out[r0 : r0 + nb * 128, :].rearrange("(j p) m -> p j m", p=128)
                if b < 2:
                    pend.append((dst, src, U))
                    continue
                if pend:
                    # scr = 0 * (batch 2's table) -> release the held batches
                    nc.vector.tensor_scalar_mul(scr[:, 0:1], U[:, 0:1], 0.0)
                    for dstp, srcp, Up in pend:
                        nc.vector.tensor_scalar_add(
                            Up[:, 1:2], Up[:, 1:2], scr[:, 0:1]
                        )
                        nc.sync.dma_start(out=dstp, in_=srcp)
                    pend = []
                # single ring: FIFO order means the engines always drain one
                # queue with deep backlog — measured 26.7 GB/s/engine (427
                # total) vs 19.7 when two queues are concurrently busy.
                nc.sync.dma_start(out=dst, in_=src)

    nc.finalize()
    return nc


def _install_trace_shim():
    """Make run_bass_kernel_spmd(trace=True) work under axon in this
    container: provide antenv.axon_hooks backed by ctypes calls into
    libaxon_pjrt.so, and skip the S3 artifact upload."""
    import contextlib
    import ctypes
    import types

    import antenv
    from concourse import bass_utils

    if getattr(antenv, "axon_hooks", None) is not None:
        return

    def _ntff_profile_via_ctypes(so_path):
        lib = ctypes.CDLL(so_path)
        if not hasattr(lib, "axon_start_nrt_profile"):
            return None
        lib.axon_start_nrt_profile.argtypes = [
            ctypes.POINTER(ctypes.c_int64),
            ctypes.c_size_t,
        ]
        lib.axon_start_nrt_profile.restype = ctypes.c_int64
        lib.axon_stop_nrt_profile.argtypes = [ctypes.c_char_p]
        lib.axon_stop_nrt_profile.restype = ctypes.c_int64

        @contextlib.contextmanager
        def _hook(output_dir, device_ids):
            import jax

            jax.devices()
            if device_ids:
                ids = (ctypes.c_int64 * len(device_ids))(*device_ids)
                rc = lib.axon_start_nrt_profile(ids, len(device_ids))
            else:
                rc = lib.axon_start_nrt_profile(None, 0)
            if rc != 0:
                raise RuntimeError(f"axon_start_nrt_profile rc={rc}")
            try:
                yield
            finally:
                n = lib.axon_stop_nrt_profile(str(output_dir).encode())
                print(f"trace shim: {n} ntff file(s) in {output_dir}", file=sys.stderr)

        return _hook

    mod = types.ModuleType("antenv.axon_hooks")
    state = {"hook": _ntff_profile_via_ctypes("/opt/axon/libaxon_pjrt.so")}
    mod.set_axon_ntff_profile_hook = lambda h: state.__setitem__("hook", h)
    mod.get_axon_ntff_profile_hook = lambda: state["hook"]
    sys.modules["antenv.axon_hooks"] = mod
    antenv.axon_hooks = mod
    bass_utils.upload_artifacts = lambda tmpdir: f"local://{tmpdir}"


def kernel(q, k, emb):
    global LAST_EXEC_TIME_NS
    trace = os.environ.get("KERNEL_TRACE", "") == "1"
    if trace:
        _install_trace_shim()

    nc = _build_nc()

    qr = np.asarray(q, dtype=np.float32).reshape(PAIRS, L, D)
    embT = np.asarray(emb, dtype=np.float32).T  # [D, NB]
    in_maps = []
    for c in range(NCORES):
        qc = qr[c * PPC : (c + 1) * PPC]  # [PPC, L, D]
        qTc = qc.transpose(2, 0, 1).reshape(D, ROWS)
        qTe = np.ascontiguousarray(
            np.concatenate([embT, qTc], axis=1).astype(ml_dtypes.bfloat16)
        )
        in_maps.append({"qTe": qTe})

    res = run_bass_kernel_spmd(nc, in_maps, core_ids=list(range(NCORES)), trace=trace)
    LAST_EXEC_TIME_NS = res.exec_time_ns

    out = np.empty((PAIRS, L, M), np.float32)
    for c in range(NCORES):
        out[c * PPC : (c + 1) * PPC] = res.results[c]["out"].reshape(PPC, L, M)
    return out.reshape(B, H, L, M)



# revision 25
# speedup vs baseline: 1.1542x; 1.1515x over previous
"""Trainium2 Bass kernel for BinaryRelativePositionEmbedding.

Math: out[b,h,l,m] = q[b,h,l,:] . rp[m,:],  rp = bits @ emb, where
bits[m,:] are the 12 two's-complement bits of position (m - L + 1).

Key identity: out[l, m] = sum_b bits[m,b] * s[l,b] with s = q @ emb^T
(rank 12).  The pattern v(m) = (m - (L-1)) & 4095 ranges over all 12-bit
values except 2048, so each row-tile of the output is a subset-sum table
over the 12 per-row scalars s[l, :], built with doubling steps on the
vector engine.  The table is laid out rotated by 2048 so the final
output row is the single contiguous slice U[:, 1:4096]:
    U[:, 2048+w] = subset-sum of bits 0..10 over w   (w in [0,2048))
    U[:, c]      = U[:, 2048+c] + s_11               (c in [0,2048))
    => U[:, 1+m] = T[(m + 2049) & 4095] = out[:, m]  (m in [0,4095))
giving one 16380-byte contiguous DMA descriptor per output row.

All output batches go on the single sync HWDGE ring (inputs on the
scalar ring): with exactly one deeply-backed queue the 16 SDMA engines
drain it at ~26.7 GB/s each (~427 GB/s total, 612 ns per 16380-byte
descriptor — the SBUF-AXI per-engine ceiling).  Two concurrently-busy
queues make every engine round-robin at packet granularity and drop to
~19.7 GB/s each.  No SDMA-engine degradation over a 340 us single-ring
run.  The steady state is bistable: if the DVE builds stay ~1-2 batches
ahead of the drain (they do, 7.4 us/batch vs 9.9), dispatches always
land before the ring empties; perturbing the ramp so the DMA catches up
with the builds locks in a ~12.2 us/batch starving mode (-60 us).  The
2-tile batch structure is ALSO load-bearing: the Tile scheduler
interleaves the two tiles' build chains on the DVE, hiding each op's
completion-semaphore latency (~40% of op duration); single-tile batches
or forced serial order drop DVE throughput to 6-6.6 us/tile and make it
the pacer.  U pool bufs=3 is the max that fits SBUF (bufs=4 silently
corrupts results).

Sharding: data-parallel over the 32 (b,h) pairs, 4 per NeuronCore.
Measured: 351557 ns in the good mode (output-DMA roofline ~314 us +
~28 us guarded ramp + ~4 us tail); session baseline was ~423 us.
Caveat: the Tile scheduler's per-compile semaphore assignment is
nondeterministic and some schedules land in the ~12.2 us/batch
starving mode (~418 us) regardless of the guard, pool slack (bufs=4
via bf16 inputs was tried), or batch structure — the guard removes
the ramp-race trigger but not the compile lottery.
"""

import os
import sys

import ml_dtypes
import numpy as np

if "/opt/trn_rl_repo" not in sys.path:
    sys.path.insert(0, "/opt/trn_rl_repo")

import concourse.bass as bass  # noqa: E402
import concourse.mybir as mybir  # noqa: E402
from concourse import bacc, tile  # noqa: E402
from concourse.bass_utils import run_bass_kernel_spmd  # noqa: E402

F32 = mybir.dt.float32
BF16 = mybir.dt.bfloat16

B, H, L, D = 2, 16, 2048, 64
NB = 12                  # bits per position
M = 2 * L - 1            # 4095 relative positions
NCORES = 8
PAIRS = B * H            # 32
PPC = PAIRS // NCORES    # 4 (b,h) pairs per core
ROWS = PPC * L           # 8192 output rows per core


LAST_EXEC_TIME_NS = None


def _build_nc():
    nc = bacc.Bacc(None)
    # qTe packs embT (cols 0:12) ahead of qT (cols 12:12+ROWS) so the first
    # chunk load delivers both with a single DMA dispatch + completion
    # (a separate 3 KB embT DMA costs ~3 us of serial ramp).
    qTe = nc.declare_dram_parameter("qTe", [D, NB + ROWS], BF16, isOutput=False)
    out = nc.declare_dram_parameter("out", [ROWS, M], F32, isOutput=True)

    nt = ROWS // 128  # 64 row-tiles of 128 rows

    # Graduated input chunks (col ranges of qTe): a tiny first chunk so the
    # first matmul + table build can start early, then larger ones.  All
    # chunks are [64, csz] (8 DMA ports, ~190 GB/s) — the total 2 MB load
    # fully overlaps the early output batches.
    chunk_bounds = [(0, 268), (268, 1036), (1036, 2060),
                    (2060, 4108), (4108, 6156), (6156, 8204)]
    # matmul/copy groups (tile ranges), each within a single chunk; group 0
    # is a single tile so the first PSUM copy lands as early as possible.
    groups = [(0, 1), (1, 2), (2, 8), (8, 16), (16, 24), (24, 32),
              (32, 40), (40, 48), (48, 56), (56, 64)]

    def chunk_of(t):
        for ci, (a, b) in enumerate(chunk_bounds):
            if a <= NB + t * 128 < b:
                return ci
        raise AssertionError

    # output batches: two single-tile batches first so the first bytes hit
    # HBM as soon as tile 0's table exists, then 2-tile batches.
    batches = [[0], [1]] + [[t, t + 1] for t in range(2, nt, 2)]

    with tile.TileContext(nc) as tc:
        with (
            tc.tile_pool(name="const", bufs=1) as cpool,
            tc.tile_pool(name="psum", bufs=3, space="PSUM") as ppool,
            tc.tile_pool(name="tab", bufs=4) as tpool,
        ):
            s_sb = cpool.tile([128, nt * NB], F32)
            scr = cpool.tile([128, 1], F32, name="scr", tag="scr")
            qt_chunks = [
                cpool.tile([D, b - a], BF16, name=f"qt{ci}", tag=f"qt{ci}")
                for ci, (a, b) in enumerate(chunk_bounds)
            ]
            embt_sb = qt_chunks[0][:, 0:NB]

            # All inputs on the scalar ring: the sync ring is reserved for
            # output batches so its queue is the only deep backlog and the
            # SDMA engines never round-robin between two busy queues
            # (measured ~33% per-descriptor penalty when they do).
            for ci, (a, b) in enumerate(chunk_bounds):
                nc.scalar.dma_start(out=qt_chunks[ci][:], in_=qTe[:, a:b])

            # s[l, b] = q[l, :] . emb[b, :].  PSUM->SBUF copies go on the
            # ACT engine so the DVE queue is pure table builds.
            for gi, (g0, g1) in enumerate(groups):
                ng = g1 - g0
                ps = ppool.tile([128, 8 * NB], F32, name="ps", tag="ps")
                for j, t in enumerate(range(g0, g1)):
                    ci = chunk_of(t)
                    off = NB + t * 128 - chunk_bounds[ci][0]
                    nc.tensor.matmul(
                        ps[:, j * NB : (j + 1) * NB],
                        lhsT=qt_chunks[ci][:, off : off + 128],
                        rhs=embt_sb,
                        start=True,
                        stop=True,
                    )
                nc.scalar.copy(
                    out=s_sb[:, g0 * NB : g1 * NB],
                    in_=ps[:, : ng * NB],
                )

            # NOTE: the Tile scheduler interleaves ~2 adjacent tile build
            # chains on the DVE; this hides each op's completion-semaphore
            # latency (~40% of op duration) behind the other chain's ops.
            # Do NOT force strict per-tile ordering — it drops DVE
            # throughput from 3.7 to 6.6 us/tile and makes DVE the pacer.
            # The steady state is bistable: if the output stream ever
            # catches up with the builds, every dispatch arrives ~2.3 us
            # after the ring empties and the loop locks at 12.2 us/batch
            # (~315 GB/s) instead of 9.85 (~427).  Which mode a run lands
            # in is a ramp-timing race — so force it: hold the first two
            # dispatches until batch 2's table is built (a zero-add through
            # scr into a DMA-read column makes it a real data dependency),
            # guaranteeing the builds a self-sustaining 2-batch lead.
            pend = []
            for b, batch in enumerate(batches):
                nb = len(batch)
                U = tpool.tile([128, 2 * 4096], F32, name="U", tag="U")
                for j, ti in enumerate(batch):
                    sb = ti * NB
                    base = j * 4096
                    hi = base + 2048
                    nc.vector.memset(U[:, hi : hi + 1], 0.0)
                    nc.vector.tensor_copy(
                        out=U[:, hi + 1 : hi + 2], in_=s_sb[:, sb : sb + 1]
                    )
                    for k in range(1, NB - 1):
                        nc.vector.tensor_scalar_add(
                            U[:, hi + 2**k : hi + 2 ** (k + 1)],
                            U[:, hi : hi + 2**k],
                            s_sb[:, sb + k : sb + k + 1],
                        )
                    nc.vector.tensor_scalar_add(
                        U[:, base : base + 2048],
                        U[:, hi : hi + 2048],
                        s_sb[:, sb + NB - 1 : sb + NB],
                    )
                r0 = batch[0] * 128
                src = U[:, : nb * 4096].rearrange("p (j c) -> p j c", j=nb)[
                    :, :, 1:4096
                ]
                dst = out[r0 : r0 + nb * 128, :].rearrange("(j p) m -> p j m", p=128)
                if b < 3:
                    pend.append((dst, src, U))
                    continue
                if pend:
                    # scr = 0 * (batch 2's table) -> release the held batches
                    nc.vector.tensor_scalar_mul(scr[:, 0:1], U[:, 0:1], 0.0)
                    for dstp, srcp, Up in pend:
                        nc.vector.tensor_scalar_add(
                            Up[:, 1:2], Up[:, 1:2], scr[:, 0:1]
                        )
                        nc.sync.dma_start(out=dstp, in_=srcp)
                    pend = []
                # single ring: FIFO order means the engines always drain one
                # queue with deep backlog — measured 26.7 GB/s/engine (427
                # total) vs 19.7 when two queues are concurrently busy.
                nc.sync.dma_start(out=dst, in_=src)

    nc.finalize()
    return nc


def _install_trace_shim():
    """Make run_bass_kernel_spmd(trace=True) work under axon in this
    container: provide antenv.axon_hooks backed by ctypes calls into
    libaxon_pjrt.so, and skip the S3 artifact upload."""
    import contextlib
    import ctypes
    import types

    import antenv
    from concourse import bass_utils

    if getattr(antenv, "axon_hooks", None) is not None:
        return

    def _ntff_profile_via_ctypes(so_path):
        lib = ctypes.CDLL(so_path)
        if not hasattr(lib, "axon_start_nrt_profile"):
            return None
        lib.axon_start_nrt_profile.argtypes = [
            ctypes.POINTER(ctypes.c_int64),
            ctypes.c_size_t,
        ]
        lib.axon_start_nrt_profile.restype = ctypes.c_int64
        lib.axon_stop_nrt_profile.argtypes = [ctypes.c_char_p]
        lib.axon_stop_nrt_profile.restype = ctypes.c_int64

        @contextlib.contextmanager
        def _hook(output_dir, device_ids):
            import jax

            jax.devices()
            if device_ids:
                ids = (ctypes.c_int64 * len(device_ids))(*device_ids)
                rc = lib.axon_start_nrt_profile(ids, len(device_ids))
            else:
                rc = lib.axon_start_nrt_profile(None, 0)
            if rc != 0:
                raise RuntimeError(f"axon_start_nrt_profile rc={rc}")
            try:
                yield
            finally:
                n = lib.axon_stop_nrt_profile(str(output_dir).encode())
                print(f"trace shim: {n} ntff file(s) in {output_dir}", file=sys.stderr)

        return _hook

    mod = types.ModuleType("antenv.axon_hooks")
    state = {"hook": _ntff_profile_via_ctypes("/opt/axon/libaxon_pjrt.so")}
    mod.set_axon_ntff_profile_hook = lambda h: state.__setitem__("hook", h)
    mod.get_axon_ntff_profile_hook = lambda: state["hook"]
    sys.modules["antenv.axon_hooks"] = mod
    antenv.axon_hooks = mod
    bass_utils.upload_artifacts = lambda tmpdir: f"local://{tmpdir}"


def kernel(q, k, emb):
    global LAST_EXEC_TIME_NS
    trace = os.environ.get("KERNEL_TRACE", "") == "1"
    if trace:
        _install_trace_shim()

    nc = _build_nc()

    qr = np.asarray(q, dtype=np.float32).reshape(PAIRS, L, D)
    embT = np.asarray(emb, dtype=np.float32).T  # [D, NB]
    in_maps = []
    for c in range(NCORES):
        qc = qr[c * PPC : (c + 1) * PPC]  # [PPC, L, D]
        qTc = qc.transpose(2, 0, 1).reshape(D, ROWS)
        qTe = np.ascontiguousarray(
            np.concatenate([embT, qTc], axis=1).astype(ml_dtypes.bfloat16)
        )
        in_maps.append({"qTe": qTe})

    res = run_bass_kernel_spmd(nc, in_maps, core_ids=list(range(NCORES)), trace=trace)
    LAST_EXEC_TIME_NS = res.exec_time_ns

    out = np.empty((PAIRS, L, M), np.float32)
    for c in range(NCORES):
        out[c * PPC : (c + 1) * PPC] = res.results[c]["out"].reshape(PPC, L, M)
    return out.reshape(B, H, L, M)

